# revision 36
# baseline (speedup 1.0000x reference)
"""GAT (2-layer, PPI config) on 8 trn2 NeuronCores — sorted-tile scheme.

Math: att_unnorm[d,s] = exp(lrelu(f_src[d]+f_dst[s])) * adj[d,s].  With
x = f_src[d]+f_dst[s]:
    exp(lrelu(x)) = exp(x) * max(1, exp(-0.8x))
and after dropping the row-constant exp(f_src[d]) (softmax cancels it):
    w[d,s] = exp(f_dst[s]) * max(1, R[d] r[s]) * adj,
    R = exp(-0.8 f_src), r = exp(-0.8 f_dst).
Key identity: on any tile where x >= 0 everywhere, w = exp(f_dst)*adj, so
the matmul rhs is the RAW adjacency tile (lhs = whp = exp(f_dst)*[Wh|1]).
Where x < 0 everywhere, w = exp(0.2 f_dst) * R[d] * adj: rhs is again raw
adjacency with lhs = whpr = exp(0.2 f_dst)*[Wh|1], and the per-column
R[d] scale is applied by the HOST on the dumped accumulator.  Only tiles
straddling x = 0 need explicit per-element att (ACT fp8->bf16 upcast +
DVE tensor_scalar G + tensor_tensor mask).

Sorting source rows by f_dst and destination cols by f_src (host-side
permutation, free) makes sign-pure tiles the overwhelming majority: the
s-axis splits per 512-col chunk into a neg-prefix [0,k_c), an explicit
window [k_c,p_c), and a pos-suffix [p_c,n_st).  One SPMD program serves
all 8 cores, so (k_c, p_c) are the min/max over the per-core exact
bounds (window ~15% of tiles; correctness never depends on the split —
the explicit path is exact everywhere).

adj ships as fp8e4 (exact for 0/1), halving HBM traffic; matmuls run
mixed bf16 lhs x fp8 rhs.  PSUM holds one [128, D] f32 accumulator; each
512-col chunk = one PSUM bank runs two sequential accumulation groups
(neg then window+pos) with a mid-stream dump of the neg partial.

Sharding:
  L1: 8 cores = 4 heads x 2 column-halves (interleaved 512-blocks of the
      per-head f_src-descending order, so chunk quantiles align across
      cores).  Per core: all 8192 sources (64 tiles, f_dst-ascending),
      D=4096.
  L2: 8 cores = 4 column-quarters x 2 source-halves (d-blocks {4c+q},
      s-tiles {2t+sh} interleaved).  Per core: 32 s-tiles, D=2048.
Host: normalize num/den, apply R on neg partials, elu, un-permute.
"""

import os
import sys

sys.path.insert(0, "/opt/trn_rl_repo")

import numpy as np
import ml_dtypes

import concourse.bass as bass
import concourse.tile as tile
from concourse import bacc, mybir
from concourse.bass_utils import run_bass_kernel_spmd

BF16 = mybir.dt.bfloat16
F32 = mybir.dt.float32
FP8 = mybir.dt.float8e4
NPBF16 = ml_dtypes.bfloat16
NPFP8 = ml_dtypes.float8_e4m3

N = 8192
NFEAT = 256
NHID = 64
NHEADS = 4
NCLASS = 121
ALPHA = 0.2
N_CORES = 8
P = 128
CH = 512  # chunk width = one PSUM bank of f32

_NC_CACHE = {}
_LAST_EXEC_NS = []
_WINDOW_STATS = []


def build_sorted_kernel(n_st, D, dh, kcs, pcs, warmup=8, adj_pre=20,
                        adj_bufs=20, wsplit_t=8, win_bufs=10):
    """One attention layer shard, shared SPMD program.

    Inputs (per core):
      adjt8 [n_st*128, D] fp8   adjacency slice, rows = sorted sources,
                                cols = per-core sorted dest blocks
      whp   [128, n_st*128] bf16  pos-phase lhs: exp(f_dst)*[Wh|1], padded
      rbc   [128, D]       bf16  R = exp(-0.8 f_src[cols]), row-broadcast
      rsc   [128, n_st]    f32   r = exp(-0.8 f_dst), per s-tile column
    The neg-phase lhs whpr = whp * r is derived on-chip (DVE 4x ts).
    Output:
      out [2*(dh+1), D] bf16  rows 0:dh+1 = neg partial (host scales by
                              R), rows dh+1:  = window+pos partial.
    """
    NCH = D // CH
    assert len(kcs) == NCH and len(pcs) == NCH
    MP = 128
    max_kcs = max(kcs)
    # rbc chunks whose window opens before the gpsimd-queue bulk arrives
    # ship upfront on sync; both groups are separate contiguous DRAM
    # tensors (a column slice of a wide row-major tensor strides through
    # DRAM and gets served by only ~2 DMA engines).
    early_c = [c for c in range(NCH) if kcs[c] < 8]
    late_c = [c for c in range(NCH) if kcs[c] >= 8]
    ne = len(early_c)
    cpos = {}
    for i, c in enumerate(early_c):
        cpos[c] = (0, i)
    for i, c in enumerate(late_c):
        cpos[c] = (1, i)
    wsplit = min(wsplit_t, n_st) * MP
    nc = bacc.Bacc("TRN2", target_bir_lowering=False, debug=False,
                   num_devices=N_CORES)
    adjt_d = nc.dram_tensor("adjt8", [n_st * P, D], FP8, kind="ExternalInput")
    whpa_d = nc.dram_tensor("whpa", [P, wsplit], BF16, kind="ExternalInput")
    whpb_d = None
    if n_st * MP > wsplit:
        whpb_d = nc.dram_tensor("whpb", [P, n_st * MP - wsplit], BF16,
                                kind="ExternalInput")
    rbca_d = None
    if ne:
        rbca_d = nc.dram_tensor("rbca", [P, ne * CH], BF16,
                                kind="ExternalInput")
    rbcb_d = None
    if NCH - ne:
        rbcb_d = nc.dram_tensor("rbcb", [P, (NCH - ne) * CH], BF16,
                                kind="ExternalInput")
    rsc_d = nc.dram_tensor("rsc", [P, n_st], F32, kind="ExternalInput")
    # chunk-major f32 output: each 512-col dump is a contiguous DRAM
    # block with 2KB lines — DMAs with >=2KB per-partition lines stripe
    # across all 16 DMA engines, 1KB lines get served by only 2.
    out_d = nc.dram_tensor("out", [NCH * 2 * (dh + 1), CH], F32,
                           kind="ExternalOutput")

    with tile.TileContext(nc) as tc:
        with (
            tc.tile_pool(name="const", bufs=1) as cpool,
            tc.tile_pool(name="adj", bufs=adj_bufs) as apool,
            tc.tile_pool(name="adjb", bufs=win_bufs) as bpool,
            tc.tile_pool(name="g", bufs=win_bufs) as gpool,
            tc.tile_pool(name="att", bufs=win_bufs) as attpool,
            tc.tile_pool(name="acc", bufs=1,
                         space=bass.MemorySpace.PSUM) as pspool,
        ):
            # DMA routing: sync's queue carries ONLY the adjacency stream
            # (its wire rate ~1.4us/tile barely beats PE's ~1.75us/s-tile
            # consumption; a starved PE also drops its HAM pstate).  All
            # bulk side tensors ride gpsimd's separate DMA queue, outputs
            # go out via scalar/gpsimd.
            rsc = cpool.tile([P, n_st], F32)
            nc.sync.dma_start(rsc[:], rsc_d[:])
            whp_a = cpool.tile([P, wsplit], BF16)
            nc.sync.dma_start(whp_a[:], whpa_d[:])
            rbc_a = None
            if ne:
                rbc_a = cpool.tile([P, ne * CH], BF16)
                nc.sync.dma_start(rbc_a[:], rbca_d[:])
            # bulk side tensors ride the scalar engine's HWDGE queue —
            # off the adjacency queue, and scalar's sequencer boots much
            # faster than gpsimd's (~6us of Q7 init)
            whp_b = None
            if whpb_d is not None:
                whp_b = cpool.tile([P, n_st * MP - wsplit], BF16)
                nc.scalar.dma_start(whp_b[:], whpb_d[:])
            rbc_b = None
            if rbcb_d is not None:
                rbc_b = cpool.tile([P, (NCH - ne) * CH], BF16)
                nc.scalar.dma_start(rbc_b[:], rbcb_d[:])

            def rbc_slice(c):
                grp, i = cpos[c]
                t = rbc_a if grp == 0 else rbc_b
                return t[:, i * CH:(i + 1) * CH]

            adj_tiles = []

            def issue_adj(st):
                adjp = apool.tile([P, D], FP8, name=f"adj{st}", tag="adj")
                nc.sync.dma_start(adjp[:], adjt_d[st * P:(st + 1) * P, :])
                adj_tiles.append(adjp)

            for st in range(min(adj_pre, n_st)):
                issue_adj(st)

            def whp_slice(st):
                w0 = st * MP
                if w0 < wsplit:
                    return whp_a[:, w0:w0 + MP]
                return whp_b[:, w0 - wsplit:w0 - wsplit + MP]

            # neg-phase lhs derived on-chip: whpr[st] = whp[st] * r[st]
            # (one DVE 4x ts per s-tile, emitted in-loop to keep DVE order)
            whpr_tiles = [None] * n_st

            acc = pspool.tile([MP, D], F32, tag="acc")
            negstage = cpool.tile([dh + 1, D], F32)
            posstage = cpool.tile([dh + 1, D], F32)

            if warmup:
                # Short matmul burst so the PE HAM un-throttles toward
                # 2.4 GHz; two alternating PSUM regions so the burst
                # pipelines instead of serializing on one group.
                dmy = cpool.tile([P, 1024], BF16)
                nc.vector.memset(dmy[:], 0.0)
                for w in range(warmup):
                    j0 = (w % 2) * 512
                    nc.tensor.matmul(acc[:, j0:j0 + 512], dmy[:, 0:MP],
                                     dmy[:, j0:j0 + 512],
                                     start=True, stop=True)

            for st in range(n_st):
                if st + adj_pre < n_st:
                    issue_adj(st + adj_pre)
                adj = adj_tiles[st]
                if st < max_kcs:
                    t = cpool.tile([P, MP], BF16, name=f"whpr{st}")
                    nc.vector.tensor_scalar_mul(t[:], whp_slice(st),
                                                rsc[:, st:st + 1])
                    whpr_tiles[st] = t
                # explicit-att window tiles: ACT upcast + DVE ts/tt
                atts = {}
                for c in range(NCH):
                    if kcs[c] <= st < pcs[c]:
                        sl = slice(c * CH, (c + 1) * CH)
                        adjb = bpool.tile([P, CH], BF16, tag="adjb")
                        nc.scalar.activation(
                            adjb[:], adj[:, sl],
                            mybir.ActivationFunctionType.Copy)
                        g = gpool.tile([P, CH], BF16, tag="g")
                        nc.vector.tensor_scalar(
                            g[:], rbc_slice(c), rsc[:, st:st + 1], 1.0,
                            mybir.AluOpType.mult, mybir.AluOpType.max)
                        att = attpool.tile([P, CH], BF16, tag="att")
                        nc.vector.tensor_tensor(att[:], g[:], adjb[:],
                                                mybir.AluOpType.mult)
                        atts[c] = att
                # matmuls: pure pos (lhs=whp), pure neg (lhs=whpr), then
                # window matmuls last so PE has independent work while the
                # DVE/ACT att chain lands.  On the final s-tile, emit each
                # chunk's dump right after its closing matmul so the copy
                # and output DMA overlap the remaining chunks' matmuls.
                def dump(c, stage, part, final=False):
                    sl = slice(c * CH, (c + 1) * CH)
                    if c % 2 == 0:
                        nc.vector.tensor_copy(stage[:, sl],
                                              acc[0:dh + 1, sl])
                    else:
                        nc.scalar.copy(stage[:, sl], acc[0:dh + 1, sl])
                    # final dumps: spread DMA issue over 3 engines so the
                    # tail isn't serialized on one sequencer
                    eng = ([nc.sync, nc.scalar, nc.gpsimd]
                           [c % 3] if final else nc.scalar)
                    row0 = (c * 2 + part) * (dh + 1)
                    eng.dma_start(out_d[row0:row0 + dh + 1, :],
                                  stage[:, sl])

                if st == n_st - 1:
                    for c in range(NCH):
                        sl = slice(c * CH, (c + 1) * CH)
                        if st < kcs[c]:
                            nc.tensor.matmul(acc[:, sl], whpr_tiles[st][:],
                                             adj[:, sl], start=(st == 0),
                                             stop=True)
                            dump(c, negstage, 0, final=True)
                        else:
                            nc.tensor.matmul(acc[:, sl], whp_slice(st),
                                             atts[c][:] if c in atts
                                             else adj[:, sl],
                                             start=(st == kcs[c]),
                                             stop=True)
                            dump(c, posstage, 1, final=True)
                    continue
                for c in range(NCH):
                    if st < kcs[c] or c in atts:
                        continue
                    sl = slice(c * CH, (c + 1) * CH)
                    nc.tensor.matmul(acc[:, sl], whp_slice(st), adj[:, sl],
                                     start=(st == kcs[c]),
                                     stop=False)
                for c in range(NCH):
                    if st >= kcs[c]:
                        continue
                    sl = slice(c * CH, (c + 1) * CH)
                    nc.tensor.matmul(acc[:, sl], whpr_tiles[st][:],
                                     adj[:, sl],
                                     start=(st == 0),
                                     stop=(st == kcs[c] - 1))
                for c in sorted(atts):
                    sl = slice(c * CH, (c + 1) * CH)
                    nc.tensor.matmul(acc[:, sl], whp_slice(st),
                                     atts[c][:],
                                     start=(st == kcs[c]),
                                     stop=False)
                # neg-partial dumps as each chunk's neg group closes
                for c in range(NCH):
                    if 0 < kcs[c] <= n_st - 1 and st == kcs[c] - 1:
                        dump(c, negstage, 0)

    nc.compile()
    return nc


def _get_kernel(n_st, D, dh, kcs, pcs, **kw):
    key = (n_st, D, dh, tuple(kcs), tuple(pcs), tuple(sorted(kw.items())))
    if key not in _NC_CACHE:
        _NC_CACHE[key] = build_sorted_kernel(n_st, D, dh, list(kcs),
                                             list(pcs), **kw)
    return _NC_CACHE[key]


def _classify(f_src_cols, tmin, tmax, n_st):
    """Per 512-col chunk: (n, p) = end of all-neg prefix / start of
    all-pos suffix, given sorted s-tile f_dst mins/maxes."""
    res = []
    for c0 in range(0, len(f_src_cols), CH):
        fs = f_src_cols[c0:c0 + CH]
        T1, T2 = -fs.max(), -fs.min()
        nn = int((tmax < T1).sum())
        p_arr = np.nonzero(tmin >= T2)[0]
        pp = int(p_arr[0]) if len(p_arr) else n_st
        res.append((nn, max(pp, nn)))
    return res


def _prep_lhs(Wh_s, f_dst_s, dh, n_st):
    """whp stationary buffer from sorted-row Wh and f_dst."""
    MP = 128
    v = np.exp(f_dst_s).astype(np.float32)
    aug = np.concatenate([Wh_s, np.ones((len(f_dst_s), 1), np.float32)],
                         axis=1)  # [S, dh+1]
    whp = np.zeros((P, n_st * MP), dtype=NPBF16)
    a1 = (aug * v[:, None]).astype(NPBF16).reshape(n_st, P, dh + 1)
    for st in range(n_st):
        whp[:, st * MP:st * MP + dh + 1] = a1[st]
    return whp


def _launch(nc, in_maps):
    trace = bool(os.environ.get("GAT_TRACE"))
    res = run_bass_kernel_spmd(nc, in_maps, list(range(N_CORES)),
                               trace=trace)
    if trace:
        _LAST_EXEC_NS.append(res.exec_time_ns)
    return [res.results[c]["out"] for c in range(N_CORES)]


def _run_layer(adjT8, Wh_heads, f_src_heads, f_dst_heads, dh, core_specs,
               n_st, D, **kw):
    """core_specs: list of (head, d_cols, s_rows_sorted_idx) per core.
    Returns per-core (neg, pos) accumulators plus shared kcs/pcs."""
    n_cores = len(core_specs)
    cls = []
    for (h, d_cols, s_idx) in core_specs:
        fd = f_dst_heads[h][s_idx]
        tmin = fd.reshape(n_st, P).min(1)
        tmax = fd.reshape(n_st, P).max(1)
        cls.append(_classify(f_src_heads[h][d_cols], tmin, tmax, n_st))
    cls = np.array(cls)  # [cores, NCH, 2]
    kcs = cls[:, :, 0].min(0)
    pcs = cls[:, :, 1].max(0)
    _WINDOW_STATS.append(int((pcs - kcs).sum()))
    nc = _get_kernel(n_st, D, dh, kcs, pcs, **kw)
    wsplit_t = kw.get("wsplit_t", 8)
    wsplit = min(wsplit_t, n_st) * 128
    NCH = D // CH
    early_c = [c for c in range(NCH) if kcs[c] < 8]
    late_c = [c for c in range(NCH) if kcs[c] >= 8]
    in_maps = []
    for (h, d_cols, s_idx) in core_specs:
        whp = _prep_lhs(Wh_heads[h][s_idx], f_dst_heads[h][s_idx],
                        dh, n_st)
        rbc = np.broadcast_to(
            np.exp(-(1 - ALPHA) * f_src_heads[h][d_cols]).astype(NPBF16),
            (P, D)).copy()
        rsc = np.ascontiguousarray(
            np.exp(-(1 - ALPHA) * f_dst_heads[h][s_idx])
            .astype(np.float32).reshape(n_st, P).T)
        adjt8 = adjT8[np.ix_(s_idx, d_cols)]
        im = {"adjt8": np.ascontiguousarray(adjt8),
              "whpa": np.ascontiguousarray(whp[:, :wsplit]),
              "rsc": rsc}
        if n_st * 128 > wsplit:
            im["whpb"] = np.ascontiguousarray(whp[:, wsplit:])
        if early_c:
            im["rbca"] = np.ascontiguousarray(np.concatenate(
                [rbc[:, c * CH:(c + 1) * CH] for c in early_c], axis=1))
        if late_c:
            im["rbcb"] = np.ascontiguousarray(np.concatenate(
                [rbc[:, c * CH:(c + 1) * CH] for c in late_c], axis=1))
        in_maps.append(im)
    outs = _launch(nc, in_maps)
    return outs, kcs, pcs


def _combine(out, kcs, pcs, f_src_cols, dh, n_st):
    """raw = pos + R*neg per chunk, honoring which groups exist.
    out is chunk-major: [NCH, 2, dh+1, CH]."""
    NCH = len(kcs)
    o = out.astype(np.float32).reshape(NCH, 2, dh + 1, CH)
    raw = np.zeros((dh + 1, NCH * CH), np.float32)
    R = np.exp(-(1 - ALPHA) * f_src_cols).astype(np.float32)
    for c in range(NCH):
        sl = slice(c * CH, (c + 1) * CH)
        if kcs[c] >= n_st:
            raw[:, sl] = R[sl][None, :] * o[c, 0]
        elif kcs[c] > 0:
            raw[:, sl] = o[c, 1] + R[sl][None, :] * o[c, 0]
        else:
            raw[:, sl] = o[c, 1]
    return raw


def kernel(x, adj, Ws, a_heads, W_out, a_out):
    _LAST_EXEC_NS.clear()
    _WINDOW_STATS.clear()
    x = np.asarray(x, dtype=np.float32)
    adj = np.asarray(adj, dtype=np.float32)
    Ws = np.asarray(Ws, dtype=np.float32)
    a_heads = np.asarray(a_heads, dtype=np.float32)
    W_out = np.asarray(W_out, dtype=np.float32)
    a_out = np.asarray(a_out, dtype=np.float32)

    # adjT8[s, d] = adj[d, s] as fp8 (exact 0/1), shared by both layers
    adjT8 = adj.T.astype(NPFP8)

    # ---- Layer 1: 4 heads x 2 col-halves, full s ----
    Wh = [x @ Ws[h] for h in range(NHEADS)]
    f_src = [Wh[h] @ a_heads[h][:NHID] for h in range(NHEADS)]
    f_dst = [Wh[h] @ a_heads[h][NHID:] for h in range(NHEADS)]
    d_orders = [np.argsort(-f_src[h]) for h in range(NHEADS)]
    s_orders = [np.argsort(f_dst[h]) for h in range(NHEADS)]
    D1, n_st1 = N // 2, N // P
    core_specs = []
    for c in range(N_CORES):
        h, j = c // 2, c % 2
        blocks = [d_orders[h][(2 * cc + j) * CH:(2 * cc + j + 1) * CH]
                  for cc in range(D1 // CH)]
        core_specs.append((h, np.concatenate(blocks), s_orders[h]))
    outs, kcs, pcs = _run_layer(adjT8, Wh, f_src, f_dst, NHID, core_specs,
                                n_st1, D1)
    h_cat = np.empty((N, NHEADS * NHID), dtype=np.float32)
    for c in range(N_CORES):
        h, d_cols = core_specs[c][0], core_specs[c][1]
        raw = _combine(outs[c], kcs, pcs, f_src[h][d_cols], NHID, n_st1)
        ht = (raw[0:NHID, :] / raw[NHID, :][None, :]).T  # [D1, NHID]
        ht = np.where(ht > 0, ht, np.expm1(np.minimum(ht, 0)))
        h_cat[d_cols, h * NHID:(h + 1) * NHID] = ht

    # ---- Layer 2: 2 col-halves x 4 s-quarters, 1 head ----
    Wh2 = h_cat @ W_out
    f_src2 = Wh2 @ a_out[:NCLASS]
    f_dst2 = Wh2 @ a_out[NCLASS:]
    d_order2 = np.argsort(-f_src2)
    s_order2 = np.argsort(f_dst2)
    D2, n_st2 = N // 2, N // 4 // P
    core_specs2 = []
    for c in range(N_CORES):
        j, sh = c // 4, c % 4
        blocks = [d_order2[(2 * cc + j) * CH:(2 * cc + j + 1) * CH]
                  for cc in range(D2 // CH)]
        s_idx = np.concatenate(
            [s_order2[(4 * t + sh) * P:(4 * t + sh + 1) * P]
             for t in range(n_st2)])
        core_specs2.append((0, np.concatenate(blocks), s_idx))
    outs2, kcs2, pcs2 = _run_layer(adjT8, [Wh2], [f_src2], [f_dst2],
                                   NCLASS, core_specs2, n_st2, D2)
    out = np.empty((N, NCLASS), dtype=np.float32)
    for j in range(2):
        d_cols = core_specs2[4 * j][1]
        raw = sum(_combine(outs2[4 * j + sh], kcs2, pcs2,
                           f_src2[d_cols], NCLASS, n_st2)
                  for sh in range(4))
        out[d_cols, :] = (raw[0:NCLASS, :] / raw[NCLASS, :][None, :]).T
    return out


# revision 39
# speedup vs baseline: 1.1722x; 1.1722x over previous
"""GAT (2-layer, PPI config) on 8 trn2 NeuronCores — sorted-tile scheme.

Math: att_unnorm[d,s] = exp(lrelu(f_src[d]+f_dst[s])) * adj[d,s].  With
x = f_src[d]+f_dst[s]:
    exp(lrelu(x)) = exp(x) * max(1, exp(-0.8x))
and after dropping the row-constant exp(f_src[d]) (softmax cancels it):
    w[d,s] = exp(f_dst[s]) * max(1, R[d] r[s]) * adj,
    R = exp(-0.8 f_src), r = exp(-0.8 f_dst).
Key identity: on any tile where x >= 0 everywhere, w = exp(f_dst)*adj, so
the matmul rhs is the RAW adjacency tile (lhs = whp = exp(f_dst)*[Wh|1]).
Where x < 0 everywhere, w = exp(0.2 f_dst) * R[d] * adj: rhs is again raw
adjacency with lhs = whpr = exp(0.2 f_dst)*[Wh|1], and the per-column
R[d] scale is applied by the HOST on the dumped accumulator.  Only tiles
straddling x = 0 need explicit per-element att (ACT fp8->bf16 upcast +
DVE tensor_scalar G + tensor_tensor mask).

Sorting source rows by f_dst and destination cols by f_src (host-side
permutation, free) makes sign-pure tiles the overwhelming majority: the
s-axis splits per 512-col chunk into a neg-prefix [0,k_c), an explicit
window [k_c,p_c), and a pos-suffix [p_c,n_st).  One SPMD program serves
all 8 cores, so (k_c, p_c) are the min/max over the per-core exact
bounds (window ~15% of tiles; correctness never depends on the split —
the explicit path is exact everywhere).

adj ships as fp8e4 (exact for 0/1), halving HBM traffic; matmuls run
mixed bf16 lhs x fp8 rhs.  PSUM holds one [128, D] f32 accumulator; each
512-col chunk = one PSUM bank runs two sequential accumulation groups
(neg then window+pos) with a mid-stream dump of the neg partial.

Sharding:
  L1: 8 cores = 4 heads x 2 column-halves (interleaved 512-blocks of the
      per-head f_src-descending order, so chunk quantiles align across
      cores).  Per core: all 8192 sources (64 tiles, f_dst-ascending),
      D=4096.
  L2: 8 cores = 4 column-quarters x 2 source-halves (d-blocks {4c+q},
      s-tiles {2t+sh} interleaved).  Per core: 32 s-tiles, D=2048.
Host: normalize num/den, apply R on neg partials, elu, un-permute.
"""

import os
import sys

sys.path.insert(0, "/opt/trn_rl_repo")

import numpy as np
import ml_dtypes

import concourse.bass as bass
import concourse.tile as tile
from concourse import bacc, mybir
from concourse.bass_utils import run_bass_kernel_spmd

BF16 = mybir.dt.bfloat16
F32 = mybir.dt.float32
FP8 = mybir.dt.float8e4
NPBF16 = ml_dtypes.bfloat16
NPFP8 = ml_dtypes.float8_e4m3

N = 8192
NFEAT = 256
NHID = 64
NHEADS = 4
NCLASS = 121
ALPHA = 0.2
N_CORES = 8
P = 128
CH = 512  # chunk width = one PSUM bank of f32

_NC_CACHE = {}
_LAST_EXEC_NS = []
_WINDOW_STATS = []


def build_sorted_kernel(n_st, D, dh, kcs, pcs, warmup=8, adj_pre=20,
                        adj_bufs=20, wsplit_t=8, win_bufs=10):
    """One attention layer shard, shared SPMD program.

    Inputs (per core):
      adjt8 [n_st*128, D] fp8   adjacency slice, rows = sorted sources,
                                cols = per-core sorted dest blocks
      whp   [128, n_st*128] bf16  pos-phase lhs: exp(f_dst)*[Wh|1], padded
      rbc   [128, D]       bf16  R = exp(-0.8 f_src[cols]), row-broadcast
      rsc   [128, n_st]    f32   r = exp(-0.8 f_dst), per s-tile column
    The neg-phase lhs whpr = whp * r is derived on-chip (DVE 4x ts).
    Output:
      out [2*(dh+1), D] bf16  rows 0:dh+1 = neg partial (host scales by
                              R), rows dh+1:  = window+pos partial.
    """
    NCH = D // CH
    assert len(kcs) == NCH and len(pcs) == NCH
    MP = 128
    max_kcs = max(kcs)
    # rbc chunks whose window opens before the gpsimd-queue bulk arrives
    # ship upfront on sync; both groups are separate contiguous DRAM
    # tensors (a column slice of a wide row-major tensor strides through
    # DRAM and gets served by only ~2 DMA engines).
    early_c = [c for c in range(NCH) if kcs[c] < 8]
    late_c = [c for c in range(NCH) if kcs[c] >= 8]
    ne = len(early_c)
    cpos = {}
    for i, c in enumerate(early_c):
        cpos[c] = (0, i)
    for i, c in enumerate(late_c):
        cpos[c] = (1, i)
    wsplit = min(wsplit_t, n_st) * MP
    nc = bacc.Bacc("TRN2", target_bir_lowering=False, debug=False,
                   num_devices=N_CORES)
    adjt_d = nc.dram_tensor("adjt8", [n_st * P, D], FP8, kind="ExternalInput")
    whpa_d = nc.dram_tensor("whpa", [P, wsplit], BF16, kind="ExternalInput")
    whpb_d = None
    if n_st * MP > wsplit:
        whpb_d = nc.dram_tensor("whpb", [P, n_st * MP - wsplit], BF16,
                                kind="ExternalInput")
    rbca_d = None
    if ne:
        rbca_d = nc.dram_tensor("rbca", [P, ne * CH], BF16,
                                kind="ExternalInput")
    rbcb_d = None
    if NCH - ne:
        rbcb_d = nc.dram_tensor("rbcb", [P, (NCH - ne) * CH], BF16,
                                kind="ExternalInput")
    rsc_d = nc.dram_tensor("rsc", [P, n_st], F32, kind="ExternalInput")
    # chunk-major bf16 output: output-write DMAs are capped at ~22GB/s
    # per queue no matter the layout, so minimize bytes and spread the
    # final dumps across the three issuing engines' queues.
    out_d = nc.dram_tensor("out", [NCH * 2 * (dh + 1), CH], BF16,
                           kind="ExternalOutput")

    with tile.TileContext(nc) as tc:
        with (
            tc.tile_pool(name="const", bufs=1) as cpool,
            tc.tile_pool(name="adj", bufs=adj_bufs) as apool,
            tc.tile_pool(name="adjb", bufs=win_bufs) as bpool,
            tc.tile_pool(name="g", bufs=win_bufs) as gpool,
            tc.tile_pool(name="att", bufs=win_bufs) as attpool,
            tc.tile_pool(name="acc", bufs=1,
                         space=bass.MemorySpace.PSUM) as pspool,
        ):
            # DMA routing: sync's queue carries ONLY the adjacency stream
            # (its wire rate ~1.4us/tile barely beats PE's ~1.75us/s-tile
            # consumption; a starved PE also drops its HAM pstate).  All
            # bulk side tensors ride gpsimd's separate DMA queue, outputs
            # go out via scalar/gpsimd.
            rsc = cpool.tile([P, n_st], F32)
            nc.sync.dma_start(rsc[:], rsc_d[:])
            whp_a = cpool.tile([P, wsplit], BF16)
            nc.sync.dma_start(whp_a[:], whpa_d[:])
            rbc_a = None
            if ne:
                rbc_a = cpool.tile([P, ne * CH], BF16)
                nc.sync.dma_start(rbc_a[:], rbca_d[:])
            # bulk side tensors ride the scalar engine's HWDGE queue —
            # off the adjacency queue, and scalar's sequencer boots much
            # faster than gpsimd's (~6us of Q7 init)
            whp_b = None
            if whpb_d is not None:
                whp_b = cpool.tile([P, n_st * MP - wsplit], BF16)
                nc.scalar.dma_start(whp_b[:], whpb_d[:])
            rbc_b = None
            if rbcb_d is not None:
                rbc_b = cpool.tile([P, (NCH - ne) * CH], BF16)
                nc.scalar.dma_start(rbc_b[:], rbcb_d[:])

            def rbc_slice(c):
                grp, i = cpos[c]
                t = rbc_a if grp == 0 else rbc_b
                return t[:, i * CH:(i + 1) * CH]

            adj_tiles = []

            def issue_adj(st):
                adjp = apool.tile([P, D], FP8, name=f"adj{st}", tag="adj")
                nc.sync.dma_start(adjp[:], adjt_d[st * P:(st + 1) * P, :])
                adj_tiles.append(adjp)

            for st in range(min(adj_pre, n_st)):
                issue_adj(st)

            def whp_slice(st):
                w0 = st * MP
                if w0 < wsplit:
                    return whp_a[:, w0:w0 + MP]
                return whp_b[:, w0 - wsplit:w0 - wsplit + MP]

            # neg-phase lhs derived on-chip: whpr[st] = whp[st] * r[st]
            # (one DVE 4x ts per s-tile, emitted in-loop to keep DVE order)
            whpr_tiles = [None] * n_st

            acc = pspool.tile([MP, D], F32, tag="acc")
            negstage = cpool.tile([dh + 1, D], BF16)
            posstage = cpool.tile([dh + 1, D], BF16)

            if warmup:
                # Short matmul burst so the PE HAM un-throttles toward
                # 2.4 GHz; two alternating PSUM regions so the burst
                # pipelines instead of serializing on one group.
                dmy = cpool.tile([P, 1024], BF16)
                nc.vector.memset(dmy[:], 0.0)
                for w in range(warmup):
                    j0 = (w % 2) * 512
                    nc.tensor.matmul(acc[:, j0:j0 + 512], dmy[:, 0:MP],
                                     dmy[:, j0:j0 + 512],
                                     start=True, stop=True)

            for st in range(n_st):
                if st + adj_pre < n_st:
                    issue_adj(st + adj_pre)
                adj = adj_tiles[st]
                if st < max_kcs:
                    t = cpool.tile([P, MP], BF16, name=f"whpr{st}")
                    nc.vector.tensor_scalar_mul(t[:], whp_slice(st),
                                                rsc[:, st:st + 1])
                    whpr_tiles[st] = t
                # explicit-att window tiles: ACT upcast + DVE ts/tt
                atts = {}
                for c in range(NCH):
                    if kcs[c] <= st < pcs[c]:
                        sl = slice(c * CH, (c + 1) * CH)
                        adjb = bpool.tile([P, CH], BF16, tag="adjb")
                        nc.scalar.activation(
                            adjb[:], adj[:, sl],
                            mybir.ActivationFunctionType.Copy)
                        g = gpool.tile([P, CH], BF16, tag="g")
                        nc.vector.tensor_scalar(
                            g[:], rbc_slice(c), rsc[:, st:st + 1], 1.0,
                            mybir.AluOpType.mult, mybir.AluOpType.max)
                        att = attpool.tile([P, CH], BF16, tag="att")
                        nc.vector.tensor_tensor(att[:], g[:], adjb[:],
                                                mybir.AluOpType.mult)
                        atts[c] = att
                # matmuls: pure pos (lhs=whp), pure neg (lhs=whpr), then
                # window matmuls last so PE has independent work while the
                # DVE/ACT att chain lands.  On the final s-tile, emit each
                # chunk's dump right after its closing matmul so the copy
                # and output DMA overlap the remaining chunks' matmuls.
                def dump(c, stage, part, final=False):
                    sl = slice(c * CH, (c + 1) * CH)
                    if c % 2 == 0:
                        nc.vector.tensor_copy(stage[:, sl],
                                              acc[0:dh + 1, sl])
                    else:
                        nc.scalar.copy(stage[:, sl], acc[0:dh + 1, sl])
                    # final dumps: spread DMA issue over 3 engines so the
                    # tail isn't serialized on one sequencer
                    eng = ([nc.sync, nc.scalar, nc.gpsimd]
                           [c % 3] if final else nc.scalar)
                    row0 = (c * 2 + part) * (dh + 1)
                    eng.dma_start(out_d[row0:row0 + dh + 1, :],
                                  stage[:, sl])

                if st == n_st - 1:
                    for c in range(NCH):
                        sl = slice(c * CH, (c + 1) * CH)
                        if st < kcs[c]:
                            nc.tensor.matmul(acc[:, sl], whpr_tiles[st][:],
                                             adj[:, sl], start=(st == 0),
                                             stop=True)
                            dump(c, negstage, 0, final=True)
                        else:
                            nc.tensor.matmul(acc[:, sl], whp_slice(st),
                                             atts[c][:] if c in atts
                                             else adj[:, sl],
                                             start=(st == kcs[c]),
                                             stop=True)
                            dump(c, posstage, 1, final=True)
                    continue
                for c in range(NCH):
                    if st < kcs[c] or c in atts:
                        continue
                    sl = slice(c * CH, (c + 1) * CH)
                    nc.tensor.matmul(acc[:, sl], whp_slice(st), adj[:, sl],
                                     start=(st == kcs[c]),
                                     stop=False)
                for c in range(NCH):
                    if st >= kcs[c]:
                        continue
                    sl = slice(c * CH, (c + 1) * CH)
                    nc.tensor.matmul(acc[:, sl], whpr_tiles[st][:],
                                     adj[:, sl],
                                     start=(st == 0),
                                     stop=(st == kcs[c] - 1))
                for c in sorted(atts):
                    sl = slice(c * CH, (c + 1) * CH)
                    nc.tensor.matmul(acc[:, sl], whp_slice(st),
                                     atts[c][:],
                                     start=(st == kcs[c]),
                                     stop=False)
                # neg-partial dumps as each chunk's neg group closes
                for c in range(NCH):
                    if 0 < kcs[c] <= n_st - 1 and st == kcs[c] - 1:
                        dump(c, negstage, 0)

    nc.compile()
    return nc


def _get_kernel(n_st, D, dh, kcs, pcs, **kw):
    key = (n_st, D, dh, tuple(kcs), tuple(pcs), tuple(sorted(kw.items())))
    if key not in _NC_CACHE:
        _NC_CACHE[key] = build_sorted_kernel(n_st, D, dh, list(kcs),
                                             list(pcs), **kw)
    return _NC_CACHE[key]


def _classify(f_src_cols, tmin, tmax, n_st):
    """Per 512-col chunk: (n, p) = end of all-neg prefix / start of
    all-pos suffix, given sorted s-tile f_dst mins/maxes."""
    res = []
    for c0 in range(0, len(f_src_cols), CH):
        fs = f_src_cols[c0:c0 + CH]
        T1, T2 = -fs.max(), -fs.min()
        nn = int((tmax < T1).sum())
        p_arr = np.nonzero(tmin >= T2)[0]
        pp = int(p_arr[0]) if len(p_arr) else n_st
        res.append((nn, max(pp, nn)))
    return res


def _prep_lhs(Wh_s, f_dst_s, dh, n_st):
    """whp stationary buffer from sorted-row Wh and f_dst."""
    MP = 128
    v = np.exp(f_dst_s).astype(np.float32)
    aug = np.concatenate([Wh_s, np.ones((len(f_dst_s), 1), np.float32)],
                         axis=1)  # [S, dh+1]
    whp = np.zeros((P, n_st * MP), dtype=NPBF16)
    a1 = (aug * v[:, None]).astype(NPBF16).reshape(n_st, P, dh + 1)
    for st in range(n_st):
        whp[:, st * MP:st * MP + dh + 1] = a1[st]
    return whp


def _launch(nc, in_maps):
    trace = bool(os.environ.get("GAT_TRACE"))
    res = run_bass_kernel_spmd(nc, in_maps, list(range(N_CORES)),
                               trace=trace)
    if trace:
        _LAST_EXEC_NS.append(res.exec_time_ns)
    return [res.results[c]["out"] for c in range(N_CORES)]


def _run_layer(adjT8, Wh_heads, f_src_heads, f_dst_heads, dh, core_specs,
               n_st, D, **kw):
    """core_specs: list of (head, d_cols, s_rows_sorted_idx) per core.
    Returns per-core (neg, pos) accumulators plus shared kcs/pcs."""
    n_cores = len(core_specs)
    cls = []
    for (h, d_cols, s_idx) in core_specs:
        fd = f_dst_heads[h][s_idx]
        tmin = fd.reshape(n_st, P).min(1)
        tmax = fd.reshape(n_st, P).max(1)
        cls.append(_classify(f_src_heads[h][d_cols], tmin, tmax, n_st))
    cls = np.array(cls)  # [cores, NCH, 2]
    kcs = cls[:, :, 0].min(0)
    pcs = cls[:, :, 1].max(0)
    _WINDOW_STATS.append(int((pcs - kcs).sum()))
    nc = _get_kernel(n_st, D, dh, kcs, pcs, **kw)
    wsplit_t = kw.get("wsplit_t", 8)
    wsplit = min(wsplit_t, n_st) * 128
    NCH = D // CH
    early_c = [c for c in range(NCH) if kcs[c] < 8]
    late_c = [c for c in range(NCH) if kcs[c] >= 8]
    in_maps = []
    for (h, d_cols, s_idx) in core_specs:
        whp = _prep_lhs(Wh_heads[h][s_idx], f_dst_heads[h][s_idx],
                        dh, n_st)
        rbc = np.broadcast_to(
            np.exp(-(1 - ALPHA) * f_src_heads[h][d_cols]).astype(NPBF16),
            (P, D)).copy()
        rsc = np.ascontiguousarray(
            np.exp(-(1 - ALPHA) * f_dst_heads[h][s_idx])
            .astype(np.float32).reshape(n_st, P).T)
        adjt8 = adjT8[np.ix_(s_idx, d_cols)]
        im = {"adjt8": np.ascontiguousarray(adjt8),
              "whpa": np.ascontiguousarray(whp[:, :wsplit]),
              "rsc": rsc}
        if n_st * 128 > wsplit:
            im["whpb"] = np.ascontiguousarray(whp[:, wsplit:])
        if early_c:
            im["rbca"] = np.ascontiguousarray(np.concatenate(
                [rbc[:, c * CH:(c + 1) * CH] for c in early_c], axis=1))
        if late_c:
            im["rbcb"] = np.ascontiguousarray(np.concatenate(
                [rbc[:, c * CH:(c + 1) * CH] for c in late_c], axis=1))
        in_maps.append(im)
    outs = _launch(nc, in_maps)
    return outs, kcs, pcs


def _combine(out, kcs, pcs, f_src_cols, dh, n_st):
    """raw = pos + R*neg per chunk, honoring which groups exist.
    out is chunk-major: [NCH, 2, dh+1, CH]."""
    NCH = len(kcs)
    o = out.astype(np.float32).reshape(NCH, 2, dh + 1, CH)
    raw = np.zeros((dh + 1, NCH * CH), np.float32)
    R = np.exp(-(1 - ALPHA) * f_src_cols).astype(np.float32)
    for c in range(NCH):
        sl = slice(c * CH, (c + 1) * CH)
        if kcs[c] >= n_st:
            raw[:, sl] = R[sl][None, :] * o[c, 0]
        elif kcs[c] > 0:
            raw[:, sl] = o[c, 1] + R[sl][None, :] * o[c, 0]
        else:
            raw[:, sl] = o[c, 1]
    return raw


def kernel(x, adj, Ws, a_heads, W_out, a_out):
    _LAST_EXEC_NS.clear()
    _WINDOW_STATS.clear()
    x = np.asarray(x, dtype=np.float32)
    adj = np.asarray(adj, dtype=np.float32)
    Ws = np.asarray(Ws, dtype=np.float32)
    a_heads = np.asarray(a_heads, dtype=np.float32)
    W_out = np.asarray(W_out, dtype=np.float32)
    a_out = np.asarray(a_out, dtype=np.float32)

    # adjT8[s, d] = adj[d, s] as fp8 (exact 0/1), shared by both layers
    adjT8 = adj.T.astype(NPFP8)

    # ---- Layer 1: 4 heads x 2 col-halves, full s ----
    Wh = [x @ Ws[h] for h in range(NHEADS)]
    f_src = [Wh[h] @ a_heads[h][:NHID] for h in range(NHEADS)]
    f_dst = [Wh[h] @ a_heads[h][NHID:] for h in range(NHEADS)]
    d_orders = [np.argsort(-f_src[h]) for h in range(NHEADS)]
    s_orders = [np.argsort(f_dst[h]) for h in range(NHEADS)]
    D1, n_st1 = N // 2, N // P
    core_specs = []
    for c in range(N_CORES):
        h, j = c // 2, c % 2
        blocks = [d_orders[h][(2 * cc + j) * CH:(2 * cc + j + 1) * CH]
                  for cc in range(D1 // CH)]
        core_specs.append((h, np.concatenate(blocks), s_orders[h]))
    outs, kcs, pcs = _run_layer(adjT8, Wh, f_src, f_dst, NHID, core_specs,
                                n_st1, D1)
    h_cat = np.empty((N, NHEADS * NHID), dtype=np.float32)
    for c in range(N_CORES):
        h, d_cols = core_specs[c][0], core_specs[c][1]
        raw = _combine(outs[c], kcs, pcs, f_src[h][d_cols], NHID, n_st1)
        ht = (raw[0:NHID, :] / raw[NHID, :][None, :]).T  # [D1, NHID]
        ht = np.where(ht > 0, ht, np.expm1(np.minimum(ht, 0)))
        h_cat[d_cols, h * NHID:(h + 1) * NHID] = ht

    # ---- Layer 2: 4 col-quarters x 2 s-halves, 1 head ----
    Wh2 = h_cat @ W_out
    f_src2 = Wh2 @ a_out[:NCLASS]
    f_dst2 = Wh2 @ a_out[NCLASS:]
    d_order2 = np.argsort(-f_src2)
    s_order2 = np.argsort(f_dst2)
    D2, n_st2 = N // 4, N // 2 // P
    core_specs2 = []
    for c in range(N_CORES):
        q, sh = c % 4, c // 4
        blocks = [d_order2[(4 * cc + q) * CH:(4 * cc + q + 1) * CH]
                  for cc in range(D2 // CH)]
        s_idx = np.concatenate(
            [s_order2[(2 * t + sh) * P:(2 * t + sh + 1) * P]
             for t in range(n_st2)])
        core_specs2.append((0, np.concatenate(blocks), s_idx))
    outs2, kcs2, pcs2 = _run_layer(adjT8, [Wh2], [f_src2], [f_dst2],
                                   NCLASS, core_specs2, n_st2, D2)
    out = np.empty((N, NCLASS), dtype=np.float32)
    for q in range(4):
        d_cols = core_specs2[q][1]
        raw = (_combine(outs2[q], kcs2, pcs2, f_src2[d_cols], NCLASS,
                        n_st2)
               + _combine(outs2[q + 4], kcs2, pcs2, f_src2[d_cols],
                          NCLASS, n_st2))
        out[d_cols, :] = (raw[0:NCLASS, :] / raw[NCLASS, :][None, :]).T
    return out
